# revision 38
# baseline (speedup 1.0000x reference)
"""Trainium2 Bass kernel for multi-query attention with tanh-clamped softmax.

Sharding: tensor-parallel over the 8 query heads (one head per core, both
batches). K/V projections are small and replicated. The output projection is
row-parallel (each core computes a full-shape partial); the host sums the 8
partials (the unshard step for row-parallel tensor parallelism).

Stage-B layout: attention runs in 512-wide query passes so the attn@V
accumulators only hold two PSUM banks and can be double-buffered across
passes, leaving the four work banks ~3 iterations of rotation lookahead for
the PE. The attention bias enters the logits without a separate add pass:
on every fourth key-tile the PE copies the bias tile into PSUM as the
opening matmul of the sim accumulation group (tanh then reads PSUM
directly); on the rest the DVE performs the classic fused add+evacuate so
the work splits across both engines. The softmax denominator rides as a
ones-row through
attn@V and is transposed on the PE so the reciprocal runs on a [128, 4]
tile.

All matmuls run in bf16 with fp32 PSUM accumulation; layernorm statistics
and softmax normalization are computed in fp32.
"""

import functools
import os
import sys

sys.path.insert(0, "/opt/trn_rl_repo")

import numpy as np
import ml_dtypes

import concourse.bass as bass
import concourse.tile as tile
from concourse import bacc, mybir
from concourse.bass_utils import run_bass_kernel_spmd
from concourse.masks import make_identity

F32 = mybir.dt.float32
BF16 = mybir.dt.bfloat16
AF = mybir.ActivationFunctionType
ALU = mybir.AluOpType

HEADS = 8
DQK = 128
DV = 192
SCALE = 64 ** -0.5
CLAMP = 5.0
EPS = 1e-5

B = 2
N = 2048
DIM = 1536
N_CORES = 8

_LAST_STATS = {}


def build_nc(b=B, n=N, dim=DIM):
    """Build the per-core Bass graph. All cores run the same graph (SPMD)."""
    assert dim % 128 == 0 and n % 512 == 0
    DIMT = dim // 128          # dim tiles (contraction for projections)
    RT_PER_B = n // 128        # row tiles per batch
    KT = n // 128              # key tiles per batch
    QW = 512                   # query-pass width
    QH = n // QW               # query passes per batch
    CC = dim // 512            # output column chunks
    TPQ = QW // 128            # row tiles per query pass
    WCOLS = DQK + DQK + DV     # 448

    nc = bacc.Bacc("TRN2", target_bir_lowering=False)

    xT = nc.declare_dram_parameter("xT", [dim, b * n], BF16, isOutput=False)
    w_all = nc.declare_dram_parameter("w_all", [dim, WCOLS], BF16, isOutput=False)
    biasT = nc.declare_dram_parameter("biasT", [b, n, n], BF16, isOutput=False)
    w_out = nc.declare_dram_parameter("w_out", [DV, dim], BF16, isOutput=False)
    gq = nc.declare_dram_parameter("gq", [DQK, 1], F32, isOutput=False)
    gk = nc.declare_dram_parameter("gk", [DQK, 1], F32, isOutput=False)
    out = nc.declare_dram_parameter("out", [b, n, dim], BF16, isOutput=True)

    with tile.TileContext(nc) as tc:
        with (
            tc.tile_pool(name="const", bufs=1) as const,
            tc.tile_pool(name="big", bufs=1) as big,
            tc.tile_pool(name="stA", bufs=4) as sA,
            tc.tile_pool(name="stB", bufs=4) as sB,
            tc.tile_pool(name="biasp", bufs=20) as sBias,
            tc.tile_pool(name="work_ps", bufs=4, space="PSUM") as psW,
            tc.tile_pool(name="acc_ps", bufs=2, space="PSUM") as psAcc,
        ):
            # ---------------- constants ----------------
            # w_all is loaded in three region-level chunks so the first
            # projection matmuls only wait for the dim-tiles they touch.
            w_all_sb = const.tile([128, DIMT, WCOLS], BF16)
            w_all_r = w_all.rearrange("(t p) c -> p t c", p=128)
            for wp in range(3):
                lo, hi = wp * DIMT // 3, (wp + 1) * DIMT // 3
                nc.sync.dma_start(
                    out=w_all_sb[:, lo:hi, :], in_=w_all_r[:, lo:hi, :]
                )
            w_out_a = const.tile([128, dim], BF16)
            w_out_b = const.tile([64, dim], BF16)
            gq_sb = const.tile([128, 1], F32)
            nc.sync.dma_start(out=gq_sb, in_=gq[:, :])
            gk_sb = const.tile([128, 1], F32)
            nc.sync.dma_start(out=gk_sb, in_=gk[:, :])
            ident = const.tile([128, 128], BF16)
            make_identity(nc, ident)
            one_sb = const.tile([1, 1], F32)
            nc.vector.memset(one_sb, 1.0)
            eps_sb = const.tile([128, 1], F32)
            nc.vector.memset(eps_sb, EPS)

            # ---------------- resident activations ----------------
            NXC = 8                      # x chunks (columns of xT), streamed
            XCW = (b * n) // NXC
            xTr = xT.rearrange("(t p) r -> p t r", p=128)

            qT_sb = [big.tile([128, n], BF16, name=f"qT{bb}") for bb in range(b)]
            kT_sb = [big.tile([128, n], BF16, name=f"kT{bb}") for bb in range(b)]
            v_sb = [big.tile([128, KT, DV + 1], BF16, name=f"v{bb}") for bb in range(b)]
            for bb in range(b):
                nc.vector.memset(v_sb[bb][:, :, DV:DV + 1], 1.0)

            # ---------------- stage A: QKV projection + LN + transpose ----------------
            # Each row-tile's raw qkv is evacuated from PSUM to SBUF (bf16)
            # by a single ScalarE copy right after the projection matmuls, so
            # the PSUM bank frees in ~0.5us instead of being held through the
            # whole LN chain. Stats and normalization then run from SBUF.
            # Gamma (and SCALE for q) is applied on ScalarE during the
            # PSUM->SBUF copy of each transposed tile.
            RT_PER_XC = XCW // 128
            pending_tr = []

            def emit_tr(bb_, ktile_, qn_, kn_):
                qtp = psW.tile([128, 512], BF16, name="qtp", tag="w")[:, :128]
                nc.tensor.transpose(qtp, qn_, ident)
                nc.scalar.activation(
                    out=qT_sb[bb_][:, ktile_ * 128:(ktile_ + 1) * 128],
                    in_=qtp, func=AF.Copy, scale=gq_sb,
                )
                ktp = psW.tile([128, 512], BF16, name="ktp", tag="w")[:, :128]
                nc.tensor.transpose(ktp, kn_, ident)
                nc.scalar.activation(
                    out=kT_sb[bb_][:, ktile_ * 128:(ktile_ + 1) * 128],
                    in_=ktp, func=AF.Copy, scale=gk_sb,
                )

            for rt in range(b * RT_PER_B):
                bb = rt // RT_PER_B
                ktile = rt % RT_PER_B
                xc = rt // RT_PER_XC
                sub = rt % RT_PER_XC
                xoff = sub * 128
                if sub == 0:
                    xt_sb = sA.tile([128, DIMT, XCW], BF16, name="xt_sb", tag="xt",
                                    bufs=3)
                    if xc == 0:
                        # Split the first chunk into per-row-tile region DMAs
                        # so the very first matmul isn't gated on 1.5 MB.
                        for sx in range(RT_PER_XC):
                            nc.sync.dma_start(
                                out=xt_sb[:, :, sx * 128:(sx + 1) * 128],
                                in_=xTr[:, :, sx * 128:(sx + 1) * 128],
                            )
                    else:
                        nc.sync.dma_start(
                            out=xt_sb, in_=xTr[:, :, xc * XCW:(xc + 1) * XCW]
                        )

                qkv_ps = psW.tile([128, 512], F32, name="qkv_ps", tag="w")[:, :WCOLS]
                for dt_ in range(DIMT):
                    nc.tensor.matmul(
                        qkv_ps,
                        lhsT=xt_sb[:, dt_, xoff:xoff + 128],
                        rhs=w_all_sb[:, dt_, :],
                        start=(dt_ == 0),
                        stop=(dt_ == DIMT - 1),
                    )
                qkv_sb = sA.tile([128, WCOLS], BF16, name="qkv_sb")
                nc.scalar.activation(out=qkv_sb, in_=qkv_ps, func=AF.Copy)
                if len(pending_tr) >= 2:
                    emit_tr(*pending_tr.pop(0))

                # layernorm stats for the three segments (q, k, v).
                # Stats for pairs of row-tiles share one mv tile so the
                # sqrt+reciprocal run once per pair (both have ~800ns fixed
                # cost); normalization is emitted on the odd row-tile.
                segs = [(0, DQK), (DQK, DQK), (2 * DQK, DV)]
                par = rt % 2
                if par == 0:
                    mvp = sA.tile([128, 2, 3, 2], F32, name="mvp")
                stats = sA.tile([128, 3, 6], F32, name="stats")
                for si, (off, w) in enumerate(segs):
                    nc.vector.bn_stats(out=stats[:, si, :], in_=qkv_sb[:, off:off + w])
                    nc.vector.bn_aggr(out=mvp[:, par, si, :], in_=stats[:, si, :])
                if par == 0:
                    held = (bb, ktile, qkv_sb)
                    continue
                rstd = sA.tile([128, 2, 3], F32, name="rstd")
                nc.scalar.activation(rstd, mvp[:, :, :, 1], AF.Sqrt, bias=eps_sb)
                nc.vector.reciprocal(out=rstd, in_=rstd)

                for pp, (bb_, ktile_, sb_) in enumerate([held, (bb, ktile, qkv_sb)]):
                    # Normalization runs on GpSimd (SBUF-only inputs now) to
                    # keep the DVE free for stats and stage-B evacuations.
                    qn = sA.tile([128, 128], BF16, name="qn")
                    nc.gpsimd.tensor_scalar(
                        out=qn, in0=sb_[:, 0:DQK],
                        scalar1=mvp[:, pp, 0, 0:1], scalar2=rstd[:, pp, 0:1],
                        op0=ALU.subtract, op1=ALU.mult,
                    )
                    kn = sA.tile([128, 128], BF16, name="kn")
                    nc.gpsimd.tensor_scalar(
                        out=kn, in0=sb_[:, DQK:2 * DQK],
                        scalar1=mvp[:, pp, 1, 0:1], scalar2=rstd[:, pp, 1:2],
                        op0=ALU.subtract, op1=ALU.mult,
                    )
                    nc.gpsimd.tensor_scalar(
                        out=v_sb[bb_][:, ktile_, 0:DV], in0=sb_[:, 2 * DQK:WCOLS],
                        scalar1=mvp[:, pp, 2, 0:1], scalar2=rstd[:, pp, 2:3],
                        op0=ALU.subtract, op1=ALU.mult,
                    )
                    pending_tr.append((bb_, ktile_, qn, kn))

            for args in pending_tr:
                emit_tr(*args)
            pending_tr = []

            # Output-projection weights are first needed by the po drips,
            # well into stage B — load them after the stage-A DMA burst.
            nc.sync.dma_start(out=w_out_a, in_=w_out[0:128, :])
            nc.sync.dma_start(out=w_out_b, in_=w_out[128:192, :])

            # ---------------- stage B: attention + output projection ----------------
            # attn@v matmuls run one kt-iteration behind their logits so the
            # PE never blocks on the ACT chain; the previous pass's output
            # projection is drip-fed into the kt loop, with its PSUM tiles
            # evacuated (and 1/s applied) on the DVE.

            def emit_po(outUa_, outUb_, rcol_, bb_, qoff_, t):
                po_sb = sB.tile([128, dim], BF16, name="po_sb")
                for cc in range(CC):
                    po = psW.tile([128, 512], F32, name="po", tag="w")
                    nc.tensor.matmul(
                        po,
                        lhsT=outUa_[:, t * 128:(t + 1) * 128],
                        rhs=w_out_a[:, cc * 512:(cc + 1) * 512],
                        start=True, stop=False,
                    )
                    nc.tensor.matmul(
                        po,
                        lhsT=outUb_[:, t * 128:(t + 1) * 128],
                        rhs=w_out_b[:, cc * 512:(cc + 1) * 512],
                        start=False, stop=True,
                    )
                    nc.vector.tensor_scalar_mul(
                        out=po_sb[:, cc * 512:(cc + 1) * 512],
                        in0=po, scalar1=rcol_[:, t:t + 1])
                nc.sync.dma_start(
                    out=out[bb_, qoff_ + t * 128: qoff_ + (t + 1) * 128, :],
                    in_=po_sb,
                )

            # Flat software pipeline over all (batch, qpass, key-tile) steps:
            # sim at step s, tanh/exp at s (engine queues lag), attn@v at
            # s-2, so pass boundaries never drain the PE. exp runs once per
            # key-tile pair on a [128, 2*QW] tile to amortize ACT overhead.
            steps = [(bb, qh, kt)
                     for bb in range(b) for qh in range(QH) for kt in range(KT)]
            S = len(steps)
            av_q = []
            acc_tiles = {}
            e_ref = {}
            th_pair = {}
            pending_po = []

            def emit_avstage(s_):
                bb2, qh2, kt2 = steps[s_]
                if kt2 == 0:
                    acc_tiles[(bb2, qh2)] = (
                        psAcc.tile([128, 512], F32, name="accA"),
                        psAcc.tile([65, 512], F32, name="accB"),
                    )
                accA, accB = acc_tiles[(bb2, qh2)]
                pe = e_ref.pop((bb2, qh2, kt2))
                nc.tensor.matmul(
                    accA,
                    lhsT=v_sb[bb2][:, kt2, 0:128],
                    rhs=pe,
                    start=(kt2 == 0), stop=(kt2 == KT - 1),
                )
                nc.tensor.matmul(
                    accB,
                    lhsT=v_sb[bb2][:, kt2, 128:DV + 1],
                    rhs=pe,
                    start=(kt2 == 0), stop=(kt2 == KT - 1),
                )
                if kt2 != KT - 1:
                    return
                # Pass complete: evacuate accumulators (fast release so the
                # next pass's attn@v can claim the banks; outUa on DVE,
                # outUb + the ones-row sums on ScalarE to split the load).
                # The PE transposes the sums into a per-partition column so
                # the reciprocal runs on a [128, 4] tile instead of a
                # pathological [1, 512] one.
                qoff2 = qh2 * QW
                outUa = sB.tile([128, QW], BF16, name="outUa")
                outUb = sB.tile([64, QW], BF16, name="outUb")
                s_sb = sB.tile([1, QW], F32, name="s_sb")
                # s_sb feeds the rcol transposes soon after; emit it first
                # and on ScalarE so it doesn't sit behind the DVE's evac
                # backlog.
                nc.scalar.activation(out=s_sb, in_=accB[64:65, :], func=AF.Copy)
                nc.scalar.activation(out=outUb, in_=accB[0:64, :], func=AF.Copy)
                nc.vector.tensor_copy(out=outUa, in_=accA)
                state = {"rcol": None}

                def mk_rcol(state_, s_sb_):
                    if state_["rcol"] is None:
                        rcol_ps = psW.tile(
                            [128, 512], F32, name="rcol_ps", tag="w")[:, :TPQ]
                        for t_ in range(TPQ):
                            nc.tensor.matmul(
                                rcol_ps[:, t_:t_ + 1],
                                lhsT=s_sb_[:, t_ * 128:(t_ + 1) * 128],
                                rhs=one_sb,
                                start=True, stop=True,
                            )
                        rcol = sB.tile([128, TPQ], F32, name="rcol")
                        nc.vector.reciprocal(out=rcol, in_=rcol_ps)
                        state_["rcol"] = rcol
                    return state_["rcol"]

                def drip(state_, outUa_, outUb_, s_sb_, bb_, qoff_, t):
                    emit_po(outUa_, outUb_, mk_rcol(state_, s_sb_),
                            bb_, qoff_, t)

                for t in range(TPQ):
                    pending_po.append(functools.partial(
                        drip, state, outUa, outUb, s_sb, bb2, qoff2, t))

            AV_SKEW = 10
            for s in range(S + AV_SKEW):
                if s < S:
                    bb, qh, kt = steps[s]
                    qoff = qh * QW
                    bias_sb = sBias.tile([128, QW], BF16, name="bias_sb")
                    nc.sync.dma_start(
                        out=bias_sb,
                        in_=biasT[bb, kt * 128:(kt + 1) * 128, qoff: qoff + QW],
                    )
                    sim_ps = psW.tile([128, 512], F32, name="sim_ps", tag="w")
                    if kt % 4 == 0:
                        # PE opens the group by copying the bias in.
                        nc.tensor.matmul(
                            sim_ps, lhsT=ident, rhs=bias_sb,
                            start=True, stop=False,
                        )
                        nc.tensor.matmul(
                            sim_ps,
                            lhsT=kT_sb[bb][:, kt * 128:(kt + 1) * 128],
                            rhs=qT_sb[bb][:, qoff: qoff + QW],
                            start=False, stop=True,
                        )
                        e_in = sim_ps
                    else:
                        # DVE folds the bias add into the PSUM evacuation.
                        nc.tensor.matmul(
                            sim_ps,
                            lhsT=kT_sb[bb][:, kt * 128:(kt + 1) * 128],
                            rhs=qT_sb[bb][:, qoff: qoff + QW],
                            start=True, stop=True,
                        )
                        t_sb = sB.tile([128, QW], F32, name="t_sb")
                        nc.vector.tensor_tensor(
                            out=t_sb, in0=sim_ps, in1=bias_sb, op=ALU.add,
                        )
                        e_in = t_sb
                    if kt % 2 == 0:
                        th_pair[(bb, qh)] = sB.tile([128, 2 * QW], F32, name="th_sb")
                    th_sb = th_pair[(bb, qh)]
                    half = (kt % 2) * QW
                    nc.scalar.activation(
                        th_sb[:, half:half + QW], e_in, AF.Tanh, scale=1.0 / CLAMP)
                    if kt % 2 == 1:
                        e2 = sB.tile([128, 2 * QW], BF16, name="e_sb", bufs=6)
                        nc.scalar.activation(e2, th_sb, AF.Exp, scale=CLAMP)
                        e_ref[(bb, qh, kt - 1)] = e2[:, 0:QW]
                        e_ref[(bb, qh, kt)] = e2[:, QW:2 * QW]
                if s >= AV_SKEW:
                    emit_avstage(s - AV_SKEW)
                if s < S and steps[s][2] % 4 == 3 and pending_po:
                    pending_po.pop(0)()

            for fn in pending_po:
                fn()

    nc.compile()
    return nc


_NC_CACHE = {}


def _get_nc(b=B, n=N, dim=DIM):
    key = (b, n, dim)
    if key not in _NC_CACHE:
        _NC_CACHE[key] = build_nc(b, n, dim)
    return _NC_CACHE[key]


def make_in_maps(x, attn_bias, w_qkv, w_out, g_q, g_k, g_v, n_cores=N_CORES):
    """Host-side shard + preprocess. Returns per-core input maps."""
    b, n, dim = x.shape
    bf = ml_dtypes.bfloat16
    xT = np.ascontiguousarray(
        x.reshape(b * n, dim).T).astype(bf)                      # [dim, b*n]
    kv_cols = np.ascontiguousarray(
        w_qkv[:, HEADS * DQK:]).astype(np.float32)               # [dim, 320]
    in_maps = []
    for c in range(n_cores):
        h = c % HEADS
        w_q_h = w_qkv[:, h * DQK:(h + 1) * DQK]
        w_all = np.concatenate([w_q_h, kv_cols], axis=1).astype(bf)  # [dim, 448]
        biasT = np.ascontiguousarray(
            attn_bias[:, h, :, :].transpose(0, 2, 1)).astype(bf)  # [b, keys, qrows]
        w_out_h = (w_out[h * DV:(h + 1) * DV, :]
                   * g_v[:, None].astype(np.float32)).astype(bf)  # [dv, dim]
        in_maps.append({
            "xT": xT,
            "w_all": w_all,
            "biasT": biasT,
            "w_out": w_out_h,
            "gq": (g_q * SCALE).astype(np.float32).reshape(DQK, 1),
            "gk": g_k.astype(np.float32).reshape(DQK, 1),
        })
    return in_maps


def kernel(x, attn_bias, w_qkv, w_out, g_q, g_k, g_v):
    x = np.asarray(x, dtype=np.float32)
    attn_bias = np.asarray(attn_bias, dtype=np.float32)
    w_qkv = np.asarray(w_qkv, dtype=np.float32)
    w_out = np.asarray(w_out, dtype=np.float32)
    g_q = np.asarray(g_q, dtype=np.float32)
    g_k = np.asarray(g_k, dtype=np.float32)
    g_v = np.asarray(g_v, dtype=np.float32)

    b, n, dim = x.shape
    nc = _get_nc(b, n, dim)
    in_maps = make_in_maps(x, attn_bias, w_qkv, w_out, g_q, g_k, g_v)
    res = run_bass_kernel_spmd(nc, in_maps, core_ids=list(range(N_CORES)),
                               trace=os.environ.get("KERNEL_TRACE", "") not in ("", "0"))
    _LAST_STATS["exec_time_ns"] = res.exec_time_ns
    _LAST_STATS["mean_exec_time_ns"] = res.mean_exec_time_ns
    _LAST_STATS["res"] = res
    out = np.zeros((b, n, dim), dtype=np.float32)
    for c in range(N_CORES):
        out += res.results[c]["out"].astype(np.float32)
    return out


# revision 39
# speedup vs baseline: 1.5565x; 1.5565x over previous
"""Trainium2 Bass kernel for multi-query attention with tanh-clamped softmax.

Sharding: tensor-parallel over the 8 query heads (one head per core, both
batches). K/V projections are small and replicated. The output projection is
row-parallel (each core computes a full-shape partial); the host sums the 8
partials (the unshard step for row-parallel tensor parallelism).

Stage-B layout: attention runs in 512-wide query passes so the attn@V
accumulators only hold two PSUM banks and can be double-buffered across
passes, leaving the four work banks ~3 iterations of rotation lookahead for
the PE. The attention bias enters the logits without a separate add pass:
on every fourth key-tile the PE copies the bias tile into PSUM as the
opening matmul of the sim accumulation group (tanh then reads PSUM
directly); on the rest the DVE performs the classic fused add+evacuate so
the work splits across both engines. The softmax denominator rides as a
ones-row through
attn@V and is transposed on the PE so the reciprocal runs on a [128, 4]
tile.

All matmuls run in bf16 with fp32 PSUM accumulation; layernorm statistics
and softmax normalization are computed in fp32.
"""

import functools
import os
import sys

sys.path.insert(0, "/opt/trn_rl_repo")

import numpy as np
import ml_dtypes

import concourse.bass as bass
import concourse.tile as tile
from concourse import bacc, mybir
from concourse.bass_utils import run_bass_kernel_spmd
from concourse.masks import make_identity

F32 = mybir.dt.float32
BF16 = mybir.dt.bfloat16
AF = mybir.ActivationFunctionType
ALU = mybir.AluOpType

HEADS = 8
DQK = 128
DV = 192
SCALE = 64 ** -0.5
CLAMP = 5.0
EPS = 1e-5

B = 2
N = 2048
DIM = 1536
N_CORES = 8

_LAST_STATS = {}


def build_nc(b=B, n=N, dim=DIM):
    """Build the per-core Bass graph. All cores run the same graph (SPMD)."""
    assert dim % 128 == 0 and n % 512 == 0
    DIMT = dim // 128          # dim tiles (contraction for projections)
    RT_PER_B = n // 128        # row tiles per batch
    KT = n // 128              # key tiles per batch
    QW = 512                   # query-pass width
    QH = n // QW               # query passes per batch
    CC = dim // 512            # output column chunks
    TPQ = QW // 128            # row tiles per query pass
    WCOLS = DQK + DQK + DV     # 448

    nc = bacc.Bacc("TRN2", target_bir_lowering=False)

    xT = nc.declare_dram_parameter("xT", [dim, b * n], BF16, isOutput=False)
    w_all = nc.declare_dram_parameter("w_all", [dim, WCOLS], BF16, isOutput=False)
    biasT = nc.declare_dram_parameter("biasT", [b, n, n], BF16, isOutput=False)
    w_out = nc.declare_dram_parameter("w_out", [DV, dim], BF16, isOutput=False)
    gq = nc.declare_dram_parameter("gq", [DQK, 1], F32, isOutput=False)
    gk = nc.declare_dram_parameter("gk", [DQK, 1], F32, isOutput=False)
    out = nc.declare_dram_parameter("out", [b, n, dim], BF16, isOutput=True)

    with tile.TileContext(nc) as tc:
        with (
            tc.tile_pool(name="const", bufs=1) as const,
            tc.tile_pool(name="big", bufs=1) as big,
            tc.tile_pool(name="stA", bufs=4) as sA,
            tc.tile_pool(name="stB", bufs=4) as sB,
            tc.tile_pool(name="biasp", bufs=20) as sBias,
            tc.tile_pool(name="work_ps", bufs=4, space="PSUM") as psW,
            tc.tile_pool(name="acc_ps", bufs=2, space="PSUM") as psAcc,
        ):
            # ---------------- constants ----------------
            # w_all is loaded in three region-level chunks so the first
            # projection matmuls only wait for the dim-tiles they touch.
            w_all_sb = const.tile([128, DIMT, WCOLS], BF16)
            w_all_r = w_all.rearrange("(t p) c -> p t c", p=128)
            for wp in range(3):
                lo, hi = wp * DIMT // 3, (wp + 1) * DIMT // 3
                nc.sync.dma_start(
                    out=w_all_sb[:, lo:hi, :], in_=w_all_r[:, lo:hi, :]
                )
            w_out_a = const.tile([128, dim], BF16)
            w_out_b = const.tile([64, dim], BF16)
            gq_sb = const.tile([128, 1], F32)
            nc.sync.dma_start(out=gq_sb, in_=gq[:, :])
            gk_sb = const.tile([128, 1], F32)
            nc.sync.dma_start(out=gk_sb, in_=gk[:, :])
            ident = const.tile([128, 128], BF16)
            make_identity(nc, ident)
            one_sb = const.tile([1, 1], F32)
            nc.vector.memset(one_sb, 1.0)
            eps_sb = const.tile([128, 1], F32)
            nc.vector.memset(eps_sb, EPS)

            # ---------------- resident activations ----------------
            NXC = 8                      # x chunks (columns of xT), streamed
            XCW = (b * n) // NXC
            xTr = xT.rearrange("(t p) r -> p t r", p=128)

            qT_sb = [big.tile([128, n], BF16, name=f"qT{bb}") for bb in range(b)]
            kT_sb = [big.tile([128, n], BF16, name=f"kT{bb}") for bb in range(b)]
            v_sb = [big.tile([128, KT, DV + 1], BF16, name=f"v{bb}") for bb in range(b)]
            for bb in range(b):
                nc.vector.memset(v_sb[bb][:, :, DV:DV + 1], 1.0)

            # ---------------- stage A: QKV projection + LN + transpose ----------------
            # Each row-tile's raw qkv is evacuated from PSUM to SBUF (bf16)
            # by a single ScalarE copy right after the projection matmuls, so
            # the PSUM bank frees in ~0.5us instead of being held through the
            # whole LN chain. Stats and normalization then run from SBUF.
            # Gamma (and SCALE for q) is applied on ScalarE during the
            # PSUM->SBUF copy of each transposed tile.
            RT_PER_XC = XCW // 128
            pending_tr = []

            def emit_tr(bb_, ktile_, qn_, kn_):
                qtp = psW.tile([128, 512], BF16, name="qtp", tag="w")[:, :128]
                nc.tensor.transpose(qtp, qn_, ident)
                nc.scalar.activation(
                    out=qT_sb[bb_][:, ktile_ * 128:(ktile_ + 1) * 128],
                    in_=qtp, func=AF.Copy, scale=gq_sb,
                )
                ktp = psW.tile([128, 512], BF16, name="ktp", tag="w")[:, :128]
                nc.tensor.transpose(ktp, kn_, ident)
                nc.scalar.activation(
                    out=kT_sb[bb_][:, ktile_ * 128:(ktile_ + 1) * 128],
                    in_=ktp, func=AF.Copy, scale=gk_sb,
                )

            for rt in range(b * RT_PER_B):
                bb = rt // RT_PER_B
                ktile = rt % RT_PER_B
                xc = rt // RT_PER_XC
                sub = rt % RT_PER_XC
                xoff = sub * 128
                if sub == 0:
                    xt_sb = sA.tile([128, DIMT, XCW], BF16, name="xt_sb", tag="xt",
                                    bufs=3)
                    if xc == 0:
                        # Split the first chunk into per-row-tile region DMAs
                        # so the very first matmul isn't gated on 1.5 MB.
                        for sx in range(RT_PER_XC):
                            nc.sync.dma_start(
                                out=xt_sb[:, :, sx * 128:(sx + 1) * 128],
                                in_=xTr[:, :, sx * 128:(sx + 1) * 128],
                            )
                    else:
                        nc.sync.dma_start(
                            out=xt_sb, in_=xTr[:, :, xc * XCW:(xc + 1) * XCW]
                        )

                qkv_ps = psW.tile([128, 512], F32, name="qkv_ps", tag="w")[:, :WCOLS]
                for dt_ in range(DIMT):
                    nc.tensor.matmul(
                        qkv_ps,
                        lhsT=xt_sb[:, dt_, xoff:xoff + 128],
                        rhs=w_all_sb[:, dt_, :],
                        start=(dt_ == 0),
                        stop=(dt_ == DIMT - 1),
                    )
                qkv_sb = sA.tile([128, WCOLS], BF16, name="qkv_sb")
                nc.scalar.activation(out=qkv_sb, in_=qkv_ps, func=AF.Copy)
                if len(pending_tr) >= 2:
                    emit_tr(*pending_tr.pop(0))

                # layernorm stats for the three segments (q, k, v).
                # Stats for pairs of row-tiles share one mv tile so the
                # sqrt+reciprocal run once per pair (both have ~800ns fixed
                # cost); normalization is emitted on the odd row-tile.
                segs = [(0, DQK), (DQK, DQK), (2 * DQK, DV)]
                par = rt % 2
                if par == 0:
                    mvp = sA.tile([128, 2, 3, 2], F32, name="mvp")
                stats = sA.tile([128, 3, 6], F32, name="stats")
                for si, (off, w) in enumerate(segs):
                    nc.vector.bn_stats(out=stats[:, si, :], in_=qkv_sb[:, off:off + w])
                    nc.vector.bn_aggr(out=mvp[:, par, si, :], in_=stats[:, si, :])
                if par == 0:
                    held = (bb, ktile, qkv_sb)
                    continue
                rstd = sA.tile([128, 2, 3], F32, name="rstd")
                nc.scalar.activation(rstd, mvp[:, :, :, 1], AF.Sqrt, bias=eps_sb)
                nc.vector.reciprocal(out=rstd, in_=rstd)

                for pp, (bb_, ktile_, sb_) in enumerate([held, (bb, ktile, qkv_sb)]):
                    qn = sA.tile([128, 128], BF16, name="qn")
                    nc.vector.tensor_scalar(
                        out=qn, in0=sb_[:, 0:DQK],
                        scalar1=mvp[:, pp, 0, 0:1], scalar2=rstd[:, pp, 0:1],
                        op0=ALU.subtract, op1=ALU.mult,
                    )
                    kn = sA.tile([128, 128], BF16, name="kn")
                    nc.vector.tensor_scalar(
                        out=kn, in0=sb_[:, DQK:2 * DQK],
                        scalar1=mvp[:, pp, 1, 0:1], scalar2=rstd[:, pp, 1:2],
                        op0=ALU.subtract, op1=ALU.mult,
                    )
                    nc.vector.tensor_scalar(
                        out=v_sb[bb_][:, ktile_, 0:DV], in0=sb_[:, 2 * DQK:WCOLS],
                        scalar1=mvp[:, pp, 2, 0:1], scalar2=rstd[:, pp, 2:3],
                        op0=ALU.subtract, op1=ALU.mult,
                    )
                    pending_tr.append((bb_, ktile_, qn, kn))

            for args in pending_tr:
                emit_tr(*args)
            pending_tr = []

            # Output-projection weights are first needed by the po drips,
            # well into stage B — load them after the stage-A DMA burst.
            nc.sync.dma_start(out=w_out_a, in_=w_out[0:128, :])
            nc.sync.dma_start(out=w_out_b, in_=w_out[128:192, :])

            # ---------------- stage B: attention + output projection ----------------
            # attn@v matmuls run one kt-iteration behind their logits so the
            # PE never blocks on the ACT chain; the previous pass's output
            # projection is drip-fed into the kt loop, with its PSUM tiles
            # evacuated (and 1/s applied) on the DVE.

            def emit_po(outUa_, outUb_, rcol_, bb_, qoff_, t):
                po_sb = sB.tile([128, dim], BF16, name="po_sb")
                for cc in range(CC):
                    po = psW.tile([128, 512], F32, name="po", tag="w")
                    nc.tensor.matmul(
                        po,
                        lhsT=outUa_[:, t * 128:(t + 1) * 128],
                        rhs=w_out_a[:, cc * 512:(cc + 1) * 512],
                        start=True, stop=False,
                    )
                    nc.tensor.matmul(
                        po,
                        lhsT=outUb_[:, t * 128:(t + 1) * 128],
                        rhs=w_out_b[:, cc * 512:(cc + 1) * 512],
                        start=False, stop=True,
                    )
                    nc.vector.tensor_scalar_mul(
                        out=po_sb[:, cc * 512:(cc + 1) * 512],
                        in0=po, scalar1=rcol_[:, t:t + 1])
                nc.sync.dma_start(
                    out=out[bb_, qoff_ + t * 128: qoff_ + (t + 1) * 128, :],
                    in_=po_sb,
                )

            # Flat software pipeline over all (batch, qpass, key-tile) steps:
            # sim at step s, tanh/exp at s (engine queues lag), attn@v at
            # s-2, so pass boundaries never drain the PE. exp runs once per
            # key-tile pair on a [128, 2*QW] tile to amortize ACT overhead.
            steps = [(bb, qh, kt)
                     for bb in range(b) for qh in range(QH) for kt in range(KT)]
            S = len(steps)
            av_q = []
            acc_tiles = {}
            e_ref = {}
            th_pair = {}
            pending_po = []

            def emit_avstage(s_):
                bb2, qh2, kt2 = steps[s_]
                if kt2 == 0:
                    acc_tiles[(bb2, qh2)] = (
                        psAcc.tile([128, 512], F32, name="accA"),
                        psAcc.tile([65, 512], F32, name="accB"),
                    )
                accA, accB = acc_tiles[(bb2, qh2)]
                pe = e_ref.pop((bb2, qh2, kt2))
                nc.tensor.matmul(
                    accA,
                    lhsT=v_sb[bb2][:, kt2, 0:128],
                    rhs=pe,
                    start=(kt2 == 0), stop=(kt2 == KT - 1),
                )
                nc.tensor.matmul(
                    accB,
                    lhsT=v_sb[bb2][:, kt2, 128:DV + 1],
                    rhs=pe,
                    start=(kt2 == 0), stop=(kt2 == KT - 1),
                )
                if kt2 != KT - 1:
                    return
                # Pass complete: evacuate accumulators (fast release so the
                # next pass's attn@v can claim the banks; outUa on DVE,
                # outUb + the ones-row sums on ScalarE to split the load).
                # The PE transposes the sums into a per-partition column so
                # the reciprocal runs on a [128, 4] tile instead of a
                # pathological [1, 512] one.
                qoff2 = qh2 * QW
                outUa = sB.tile([128, QW], BF16, name="outUa")
                outUb = sB.tile([64, QW], BF16, name="outUb")
                s_sb = sB.tile([1, QW], F32, name="s_sb")
                # s_sb feeds the rcol transposes soon after; emit it first
                # and on ScalarE so it doesn't sit behind the DVE's evac
                # backlog.
                nc.scalar.activation(out=s_sb, in_=accB[64:65, :], func=AF.Copy)
                nc.scalar.activation(out=outUb, in_=accB[0:64, :], func=AF.Copy)
                nc.vector.tensor_copy(out=outUa, in_=accA)
                state = {"rcol": None}

                def mk_rcol(state_, s_sb_):
                    if state_["rcol"] is None:
                        rcol_ps = psW.tile(
                            [128, 512], F32, name="rcol_ps", tag="w")[:, :TPQ]
                        for t_ in range(TPQ):
                            nc.tensor.matmul(
                                rcol_ps[:, t_:t_ + 1],
                                lhsT=s_sb_[:, t_ * 128:(t_ + 1) * 128],
                                rhs=one_sb,
                                start=True, stop=True,
                            )
                        rcol = sB.tile([128, TPQ], F32, name="rcol")
                        nc.vector.reciprocal(out=rcol, in_=rcol_ps)
                        state_["rcol"] = rcol
                    return state_["rcol"]

                def drip(state_, outUa_, outUb_, s_sb_, bb_, qoff_, t):
                    emit_po(outUa_, outUb_, mk_rcol(state_, s_sb_),
                            bb_, qoff_, t)

                for t in range(TPQ):
                    pending_po.append(functools.partial(
                        drip, state, outUa, outUb, s_sb, bb2, qoff2, t))

            AV_SKEW = 10
            for s in range(S + AV_SKEW):
                if s < S:
                    bb, qh, kt = steps[s]
                    qoff = qh * QW
                    bias_sb = sBias.tile([128, QW], BF16, name="bias_sb")
                    nc.sync.dma_start(
                        out=bias_sb,
                        in_=biasT[bb, kt * 128:(kt + 1) * 128, qoff: qoff + QW],
                    )
                    sim_ps = psW.tile([128, 512], F32, name="sim_ps", tag="w")
                    if kt % 4 == 0:
                        # PE opens the group by copying the bias in.
                        nc.tensor.matmul(
                            sim_ps, lhsT=ident, rhs=bias_sb,
                            start=True, stop=False,
                        )
                        nc.tensor.matmul(
                            sim_ps,
                            lhsT=kT_sb[bb][:, kt * 128:(kt + 1) * 128],
                            rhs=qT_sb[bb][:, qoff: qoff + QW],
                            start=False, stop=True,
                        )
                        e_in = sim_ps
                    else:
                        # DVE folds the bias add into the PSUM evacuation.
                        nc.tensor.matmul(
                            sim_ps,
                            lhsT=kT_sb[bb][:, kt * 128:(kt + 1) * 128],
                            rhs=qT_sb[bb][:, qoff: qoff + QW],
                            start=True, stop=True,
                        )
                        t_sb = sB.tile([128, QW], F32, name="t_sb")
                        nc.vector.tensor_tensor(
                            out=t_sb, in0=sim_ps, in1=bias_sb, op=ALU.add,
                        )
                        e_in = t_sb
                    if kt % 2 == 0:
                        th_pair[(bb, qh)] = sB.tile([128, 2 * QW], F32, name="th_sb")
                    th_sb = th_pair[(bb, qh)]
                    half = (kt % 2) * QW
                    nc.scalar.activation(
                        th_sb[:, half:half + QW], e_in, AF.Tanh, scale=1.0 / CLAMP)
                    if kt % 2 == 1:
                        e2 = sB.tile([128, 2 * QW], BF16, name="e_sb", bufs=6)
                        nc.scalar.activation(e2, th_sb, AF.Exp, scale=CLAMP)
                        e_ref[(bb, qh, kt - 1)] = e2[:, 0:QW]
                        e_ref[(bb, qh, kt)] = e2[:, QW:2 * QW]
                if s >= AV_SKEW:
                    emit_avstage(s - AV_SKEW)
                if s < S and steps[s][2] % 4 == 3 and pending_po:
                    pending_po.pop(0)()

            for fn in pending_po:
                fn()

    nc.compile()
    return nc


_NC_CACHE = {}


def _get_nc(b=B, n=N, dim=DIM):
    key = (b, n, dim)
    if key not in _NC_CACHE:
        _NC_CACHE[key] = build_nc(b, n, dim)
    return _NC_CACHE[key]


def make_in_maps(x, attn_bias, w_qkv, w_out, g_q, g_k, g_v, n_cores=N_CORES):
    """Host-side shard + preprocess. Returns per-core input maps."""
    b, n, dim = x.shape
    bf = ml_dtypes.bfloat16
    xT = np.ascontiguousarray(
        x.reshape(b * n, dim).T).astype(bf)                      # [dim, b*n]
    kv_cols = np.ascontiguousarray(
        w_qkv[:, HEADS * DQK:]).astype(np.float32)               # [dim, 320]
    in_maps = []
    for c in range(n_cores):
        h = c % HEADS
        w_q_h = w_qkv[:, h * DQK:(h + 1) * DQK]
        w_all = np.concatenate([w_q_h, kv_cols], axis=1).astype(bf)  # [dim, 448]
        biasT = np.ascontiguousarray(
            attn_bias[:, h, :, :].transpose(0, 2, 1)).astype(bf)  # [b, keys, qrows]
        w_out_h = (w_out[h * DV:(h + 1) * DV, :]
                   * g_v[:, None].astype(np.float32)).astype(bf)  # [dv, dim]
        in_maps.append({
            "xT": xT,
            "w_all": w_all,
            "biasT": biasT,
            "w_out": w_out_h,
            "gq": (g_q * SCALE).astype(np.float32).reshape(DQK, 1),
            "gk": g_k.astype(np.float32).reshape(DQK, 1),
        })
    return in_maps


def kernel(x, attn_bias, w_qkv, w_out, g_q, g_k, g_v):
    x = np.asarray(x, dtype=np.float32)
    attn_bias = np.asarray(attn_bias, dtype=np.float32)
    w_qkv = np.asarray(w_qkv, dtype=np.float32)
    w_out = np.asarray(w_out, dtype=np.float32)
    g_q = np.asarray(g_q, dtype=np.float32)
    g_k = np.asarray(g_k, dtype=np.float32)
    g_v = np.asarray(g_v, dtype=np.float32)

    b, n, dim = x.shape
    nc = _get_nc(b, n, dim)
    in_maps = make_in_maps(x, attn_bias, w_qkv, w_out, g_q, g_k, g_v)
    res = run_bass_kernel_spmd(nc, in_maps, core_ids=list(range(N_CORES)),
                               trace=os.environ.get("KERNEL_TRACE", "") not in ("", "0"))
    _LAST_STATS["exec_time_ns"] = res.exec_time_ns
    _LAST_STATS["mean_exec_time_ns"] = res.mean_exec_time_ns
    _LAST_STATS["res"] = res
    out = np.zeros((b, n, dim), dtype=np.float32)
    for c in range(N_CORES):
        out += res.results[c]["out"].astype(np.float32)
    return out


# revision 41
# speedup vs baseline: 1.5703x; 1.0089x over previous
"""Trainium2 Bass kernel for multi-query attention with tanh-clamped softmax.

Sharding: tensor-parallel over the 8 query heads (one head per core, both
batches). K/V projections are small and replicated. The output projection is
row-parallel (each core computes a full-shape partial); the host sums the 8
partials (the unshard step for row-parallel tensor parallelism).

Stage-B layout: attention runs in 512-wide query passes so the attn@V
accumulators only hold two PSUM banks and can be double-buffered across
passes, leaving the four work banks ~3 iterations of rotation lookahead for
the PE. The attention bias enters the logits without a separate add pass:
on every fourth key-tile the PE copies the bias tile into PSUM as the
opening matmul of the sim accumulation group (tanh then reads PSUM
directly); on the rest the DVE performs the classic fused add+evacuate so
the work splits across both engines. The softmax denominator rides as a
ones-row through
attn@V and is transposed on the PE so the reciprocal runs on a [128, 4]
tile.

All matmuls run in bf16 with fp32 PSUM accumulation; layernorm statistics
and softmax normalization are computed in fp32.
"""

import functools
import os
import sys

sys.path.insert(0, "/opt/trn_rl_repo")

import numpy as np
import ml_dtypes

import concourse.bass as bass
import concourse.tile as tile
from concourse import bacc, mybir
from concourse.bass_utils import run_bass_kernel_spmd
from concourse.masks import make_identity

F32 = mybir.dt.float32
BF16 = mybir.dt.bfloat16
AF = mybir.ActivationFunctionType
ALU = mybir.AluOpType

HEADS = 8
DQK = 128
DV = 192
SCALE = 64 ** -0.5
CLAMP = 5.0
EPS = 1e-5

B = 2
N = 2048
DIM = 1536
N_CORES = 8

_LAST_STATS = {}


def build_nc(b=B, n=N, dim=DIM):
    """Build the per-core Bass graph. All cores run the same graph (SPMD)."""
    assert dim % 128 == 0 and n % 512 == 0
    DIMT = dim // 128          # dim tiles (contraction for projections)
    RT_PER_B = n // 128        # row tiles per batch
    KT = n // 128              # key tiles per batch
    QW = 512                   # query-pass width
    QH = n // QW               # query passes per batch
    CC = dim // 512            # output column chunks
    TPQ = QW // 128            # row tiles per query pass
    WCOLS = DQK + DQK + DV     # 448

    nc = bacc.Bacc("TRN2", target_bir_lowering=False)

    xT = nc.declare_dram_parameter("xT", [dim, b * n], BF16, isOutput=False)
    w_all = nc.declare_dram_parameter("w_all", [dim, WCOLS], BF16, isOutput=False)
    biasT = nc.declare_dram_parameter("biasT", [b, n, n], BF16, isOutput=False)
    w_out = nc.declare_dram_parameter("w_out", [DV, dim], BF16, isOutput=False)
    gq = nc.declare_dram_parameter("gq", [DQK, 1], F32, isOutput=False)
    gk = nc.declare_dram_parameter("gk", [DQK, 1], F32, isOutput=False)
    out = nc.declare_dram_parameter("out", [b, n, dim], BF16, isOutput=True)

    with tile.TileContext(nc) as tc:
        with (
            tc.tile_pool(name="const", bufs=1) as const,
            tc.tile_pool(name="big", bufs=1) as big,
            tc.tile_pool(name="stA", bufs=4) as sA,
            tc.tile_pool(name="stB", bufs=4) as sB,
            tc.tile_pool(name="biasp", bufs=20) as sBias,
            tc.tile_pool(name="work_ps", bufs=4, space="PSUM") as psW,
            tc.tile_pool(name="acc_ps", bufs=2, space="PSUM") as psAcc,
        ):
            # ---------------- constants ----------------
            # w_all is loaded in three region-level chunks so the first
            # projection matmuls only wait for the dim-tiles they touch.
            w_all_sb = const.tile([128, DIMT, WCOLS], BF16)
            w_all_r = w_all.rearrange("(t p) c -> p t c", p=128)
            for wp in range(3):
                lo, hi = wp * DIMT // 3, (wp + 1) * DIMT // 3
                nc.sync.dma_start(
                    out=w_all_sb[:, lo:hi, :], in_=w_all_r[:, lo:hi, :]
                )
            w_out_a = const.tile([128, dim], BF16)
            w_out_b = const.tile([64, dim], BF16)
            gq_sb = const.tile([128, 1], F32)
            nc.sync.dma_start(out=gq_sb, in_=gq[:, :])
            gk_sb = const.tile([128, 1], F32)
            nc.sync.dma_start(out=gk_sb, in_=gk[:, :])
            ident = const.tile([128, 128], BF16)
            make_identity(nc, ident)
            one_sb = const.tile([1, 1], F32)
            nc.vector.memset(one_sb, 1.0)
            eps_sb = const.tile([128, 1], F32)
            nc.vector.memset(eps_sb, EPS)

            # ---------------- resident activations ----------------
            NXC = 8                      # x chunks (columns of xT), streamed
            XCW = (b * n) // NXC
            xTr = xT.rearrange("(t p) r -> p t r", p=128)

            qT_sb = [big.tile([128, n], BF16, name=f"qT{bb}") for bb in range(b)]
            kT_sb = [big.tile([128, n], BF16, name=f"kT{bb}") for bb in range(b)]
            v_sb = [big.tile([128, KT, DV + 1], BF16, name=f"v{bb}") for bb in range(b)]
            for bb in range(b):
                nc.vector.memset(v_sb[bb][:, :, DV:DV + 1], 1.0)

            # ---------------- stage A: QKV projection + LN + transpose ----------------
            # Each row-tile's raw qkv is evacuated from PSUM to SBUF (bf16)
            # by a single ScalarE copy right after the projection matmuls, so
            # the PSUM bank frees in ~0.5us instead of being held through the
            # whole LN chain. Stats and normalization then run from SBUF.
            # Gamma (and SCALE for q) is applied on ScalarE during the
            # PSUM->SBUF copy of each transposed tile.
            RT_PER_XC = XCW // 128
            pending_tr = []

            def emit_tr(bb_, ktile_, qn_, kn_):
                qtp = psW.tile([128, 512], BF16, name="qtp", tag="w")[:, :128]
                nc.tensor.transpose(qtp, qn_, ident)
                nc.scalar.activation(
                    out=qT_sb[bb_][:, ktile_ * 128:(ktile_ + 1) * 128],
                    in_=qtp, func=AF.Copy, scale=gq_sb,
                )
                ktp = psW.tile([128, 512], BF16, name="ktp", tag="w")[:, :128]
                nc.tensor.transpose(ktp, kn_, ident)
                nc.scalar.activation(
                    out=kT_sb[bb_][:, ktile_ * 128:(ktile_ + 1) * 128],
                    in_=ktp, func=AF.Copy, scale=gk_sb,
                )

            for rt in range(b * RT_PER_B):
                bb = rt // RT_PER_B
                ktile = rt % RT_PER_B
                xc = rt // RT_PER_XC
                sub = rt % RT_PER_XC
                xoff = sub * 128
                if sub == 0:
                    xt_sb = sA.tile([128, DIMT, XCW], BF16, name="xt_sb", tag="xt",
                                    bufs=3)
                    if xc == 0:
                        # Split the first chunk into per-row-tile region DMAs
                        # so the very first matmul isn't gated on 1.5 MB.
                        for sx in range(RT_PER_XC):
                            nc.sync.dma_start(
                                out=xt_sb[:, :, sx * 128:(sx + 1) * 128],
                                in_=xTr[:, :, sx * 128:(sx + 1) * 128],
                            )
                    else:
                        nc.sync.dma_start(
                            out=xt_sb, in_=xTr[:, :, xc * XCW:(xc + 1) * XCW]
                        )

                qkv_ps = psW.tile([128, 512], F32, name="qkv_ps", tag="w")[:, :WCOLS]
                for dt_ in range(DIMT):
                    nc.tensor.matmul(
                        qkv_ps,
                        lhsT=xt_sb[:, dt_, xoff:xoff + 128],
                        rhs=w_all_sb[:, dt_, :],
                        start=(dt_ == 0),
                        stop=(dt_ == DIMT - 1),
                    )
                qkv_sb = sA.tile([128, WCOLS], BF16, name="qkv_sb")
                nc.scalar.activation(out=qkv_sb, in_=qkv_ps, func=AF.Copy)
                if len(pending_tr) >= 2:
                    emit_tr(*pending_tr.pop(0))

                # layernorm stats for the three segments (q, k, v).
                # Stats for pairs of row-tiles share one mv tile so the
                # sqrt+reciprocal run once per pair (both have ~800ns fixed
                # cost); normalization is emitted on the odd row-tile.
                segs = [(0, DQK), (DQK, DQK), (2 * DQK, DV)]
                par = rt % 2
                if par == 0:
                    mvp = sA.tile([128, 2, 3, 2], F32, name="mvp")
                stats = sA.tile([128, 3, 6], F32, name="stats")
                for si, (off, w) in enumerate(segs):
                    nc.vector.bn_stats(out=stats[:, si, :], in_=qkv_sb[:, off:off + w])
                    nc.vector.bn_aggr(out=mvp[:, par, si, :], in_=stats[:, si, :])
                if par == 0:
                    held = (bb, ktile, qkv_sb)
                    continue
                rstd = sA.tile([128, 2, 3], F32, name="rstd")
                nc.scalar.activation(rstd, mvp[:, :, :, 1], AF.Sqrt, bias=eps_sb)
                nc.vector.reciprocal(out=rstd, in_=rstd)

                for pp, (bb_, ktile_, sb_) in enumerate([held, (bb, ktile, qkv_sb)]):
                    qn = sA.tile([128, 128], BF16, name="qn")
                    nc.vector.tensor_scalar(
                        out=qn, in0=sb_[:, 0:DQK],
                        scalar1=mvp[:, pp, 0, 0:1], scalar2=rstd[:, pp, 0:1],
                        op0=ALU.subtract, op1=ALU.mult,
                    )
                    kn = sA.tile([128, 128], BF16, name="kn")
                    nc.vector.tensor_scalar(
                        out=kn, in0=sb_[:, DQK:2 * DQK],
                        scalar1=mvp[:, pp, 1, 0:1], scalar2=rstd[:, pp, 1:2],
                        op0=ALU.subtract, op1=ALU.mult,
                    )
                    nc.vector.tensor_scalar(
                        out=v_sb[bb_][:, ktile_, 0:DV], in0=sb_[:, 2 * DQK:WCOLS],
                        scalar1=mvp[:, pp, 2, 0:1], scalar2=rstd[:, pp, 2:3],
                        op0=ALU.subtract, op1=ALU.mult,
                    )
                    pending_tr.append((bb_, ktile_, qn, kn))

            for args in pending_tr:
                emit_tr(*args)
            pending_tr = []

            # Output-projection weights are first needed by the po drips,
            # well into stage B — load them after the stage-A DMA burst.
            nc.sync.dma_start(out=w_out_a, in_=w_out[0:128, :])
            nc.sync.dma_start(out=w_out_b, in_=w_out[128:192, :])

            # ---------------- stage B: attention + output projection ----------------
            # attn@v matmuls run one kt-iteration behind their logits so the
            # PE never blocks on the ACT chain; the previous pass's output
            # projection is drip-fed into the kt loop, with its PSUM tiles
            # evacuated (and 1/s applied) on the DVE.

            def emit_po(outUa_, outUb_, rcol_, bb_, qoff_, t):
                po_sb = sB.tile([128, dim], BF16, name="po_sb")
                for cc in range(CC):
                    po = psW.tile([128, 512], F32, name="po", tag="w")
                    nc.tensor.matmul(
                        po,
                        lhsT=outUa_[:, t * 128:(t + 1) * 128],
                        rhs=w_out_a[:, cc * 512:(cc + 1) * 512],
                        start=True, stop=False,
                    )
                    nc.tensor.matmul(
                        po,
                        lhsT=outUb_[:, t * 128:(t + 1) * 128],
                        rhs=w_out_b[:, cc * 512:(cc + 1) * 512],
                        start=False, stop=True,
                    )
                    nc.vector.tensor_scalar_mul(
                        out=po_sb[:, cc * 512:(cc + 1) * 512],
                        in0=po, scalar1=rcol_[:, t:t + 1])
                nc.sync.dma_start(
                    out=out[bb_, qoff_ + t * 128: qoff_ + (t + 1) * 128, :],
                    in_=po_sb,
                )

            # Flat software pipeline over all (batch, qpass, key-tile) steps:
            # sim at step s, tanh/exp at s (engine queues lag), attn@v at
            # s-2, so pass boundaries never drain the PE. exp runs once per
            # key-tile pair on a [128, 2*QW] tile to amortize ACT overhead.
            steps = [(bb, qh, kt)
                     for bb in range(b) for qh in range(QH) for kt in range(KT)]
            S = len(steps)
            av_q = []
            acc_tiles = {}
            e_ref = {}
            th_pair = {}
            t_pair = {}
            pending_po = []

            def emit_avstage(s_):
                bb2, qh2, kt2 = steps[s_]
                if kt2 == 0:
                    acc_tiles[(bb2, qh2)] = (
                        psAcc.tile([128, 512], F32, name="accA"),
                        psAcc.tile([65, 512], F32, name="accB"),
                    )
                accA, accB = acc_tiles[(bb2, qh2)]
                pe = e_ref.pop((bb2, qh2, kt2))
                nc.tensor.matmul(
                    accA,
                    lhsT=v_sb[bb2][:, kt2, 0:128],
                    rhs=pe,
                    start=(kt2 == 0), stop=(kt2 == KT - 1),
                )
                nc.tensor.matmul(
                    accB,
                    lhsT=v_sb[bb2][:, kt2, 128:DV + 1],
                    rhs=pe,
                    start=(kt2 == 0), stop=(kt2 == KT - 1),
                )
                if kt2 != KT - 1:
                    return
                # Pass complete: evacuate accumulators (fast release so the
                # next pass's attn@v can claim the banks; outUa on DVE,
                # outUb + the ones-row sums on ScalarE to split the load).
                # The PE transposes the sums into a per-partition column so
                # the reciprocal runs on a [128, 4] tile instead of a
                # pathological [1, 512] one.
                qoff2 = qh2 * QW
                outUa = sB.tile([128, QW], BF16, name="outUa")
                outUb = sB.tile([64, QW], BF16, name="outUb")
                s_sb = sB.tile([1, QW], F32, name="s_sb")
                # s_sb feeds the rcol transposes soon after; emit it first
                # and on ScalarE so it doesn't sit behind the DVE's evac
                # backlog.
                nc.scalar.activation(out=s_sb, in_=accB[64:65, :], func=AF.Copy)
                nc.scalar.activation(out=outUb, in_=accB[0:64, :], func=AF.Copy)
                nc.vector.tensor_copy(out=outUa, in_=accA)
                state = {"rcol": None}

                def mk_rcol(state_, s_sb_):
                    if state_["rcol"] is None:
                        rcol_ps = psW.tile(
                            [128, 512], F32, name="rcol_ps", tag="w")[:, :TPQ]
                        for t_ in range(TPQ):
                            nc.tensor.matmul(
                                rcol_ps[:, t_:t_ + 1],
                                lhsT=s_sb_[:, t_ * 128:(t_ + 1) * 128],
                                rhs=one_sb,
                                start=True, stop=True,
                            )
                        rcol = sB.tile([128, TPQ], F32, name="rcol")
                        nc.vector.reciprocal(out=rcol, in_=rcol_ps)
                        state_["rcol"] = rcol
                    return state_["rcol"]

                def drip(state_, outUa_, outUb_, s_sb_, bb_, qoff_, t):
                    emit_po(outUa_, outUb_, mk_rcol(state_, s_sb_),
                            bb_, qoff_, t)

                for t in range(TPQ):
                    pending_po.append(functools.partial(
                        drip, state, outUa, outUb, s_sb, bb2, qoff2, t))

            AV_SKEW = 10
            for s in range(S + AV_SKEW):
                if s < S:
                    bb, qh, kt = steps[s]
                    qoff = qh * QW
                    bias_sb = sBias.tile([128, QW], BF16, name="bias_sb")
                    nc.sync.dma_start(
                        out=bias_sb,
                        in_=biasT[bb, kt * 128:(kt + 1) * 128, qoff: qoff + QW],
                    )
                    sim_ps = psW.tile([128, 512], F32, name="sim_ps", tag="w")
                    if kt % 4 == 0:
                        # PE opens the group by copying the bias in.
                        nc.tensor.matmul(
                            sim_ps, lhsT=ident, rhs=bias_sb,
                            start=True, stop=False,
                        )
                        nc.tensor.matmul(
                            sim_ps,
                            lhsT=kT_sb[bb][:, kt * 128:(kt + 1) * 128],
                            rhs=qT_sb[bb][:, qoff: qoff + QW],
                            start=False, stop=True,
                        )
                        e_in = sim_ps
                    else:
                        # DVE folds the bias add into the PSUM evacuation.
                        # kt pairs (2,3) share one logit tile so their tanh
                        # runs as a single 1024-wide instruction.
                        nc.tensor.matmul(
                            sim_ps,
                            lhsT=kT_sb[bb][:, kt * 128:(kt + 1) * 128],
                            rhs=qT_sb[bb][:, qoff: qoff + QW],
                            start=True, stop=True,
                        )
                        if kt % 4 == 2:
                            t_pair[(bb, qh)] = sB.tile(
                                [128, 2 * QW], F32, name="t2_sb")
                        if kt % 4 in (2, 3):
                            t_sb = t_pair[(bb, qh)][:, (kt % 2) * QW:
                                                    (kt % 2) * QW + QW]
                        else:
                            t_sb = sB.tile([128, QW], F32, name="t_sb")
                        nc.vector.tensor_tensor(
                            out=t_sb, in0=sim_ps, in1=bias_sb, op=ALU.add,
                        )
                        e_in = t_sb
                    if kt % 2 == 0:
                        th_pair[(bb, qh)] = sB.tile([128, 2 * QW], F32, name="th_sb")
                    th_sb = th_pair[(bb, qh)]
                    half = (kt % 2) * QW
                    if kt % 4 in (0, 1):
                        nc.scalar.activation(
                            th_sb[:, half:half + QW], e_in, AF.Tanh,
                            scale=1.0 / CLAMP)
                    elif kt % 4 == 3:
                        nc.scalar.activation(
                            th_sb, t_pair[(bb, qh)], AF.Tanh, scale=1.0 / CLAMP)
                    if kt % 2 == 1:
                        e2 = sB.tile([128, 2 * QW], BF16, name="e_sb", bufs=6)
                        nc.scalar.activation(e2, th_sb, AF.Exp, scale=CLAMP)
                        e_ref[(bb, qh, kt - 1)] = e2[:, 0:QW]
                        e_ref[(bb, qh, kt)] = e2[:, QW:2 * QW]
                if s >= AV_SKEW:
                    emit_avstage(s - AV_SKEW)
                if s < S and steps[s][2] % 4 == 3 and pending_po:
                    pending_po.pop(0)()

            for fn in pending_po:
                fn()

    nc.compile()
    return nc


_NC_CACHE = {}


def _get_nc(b=B, n=N, dim=DIM):
    key = (b, n, dim)
    if key not in _NC_CACHE:
        _NC_CACHE[key] = build_nc(b, n, dim)
    return _NC_CACHE[key]


def make_in_maps(x, attn_bias, w_qkv, w_out, g_q, g_k, g_v, n_cores=N_CORES):
    """Host-side shard + preprocess. Returns per-core input maps."""
    b, n, dim = x.shape
    bf = ml_dtypes.bfloat16
    xT = np.ascontiguousarray(
        x.reshape(b * n, dim).T).astype(bf)                      # [dim, b*n]
    kv_cols = np.ascontiguousarray(
        w_qkv[:, HEADS * DQK:]).astype(np.float32)               # [dim, 320]
    in_maps = []
    for c in range(n_cores):
        h = c % HEADS
        w_q_h = w_qkv[:, h * DQK:(h + 1) * DQK]
        w_all = np.concatenate([w_q_h, kv_cols], axis=1).astype(bf)  # [dim, 448]
        biasT = np.ascontiguousarray(
            attn_bias[:, h, :, :].transpose(0, 2, 1)).astype(bf)  # [b, keys, qrows]
        w_out_h = (w_out[h * DV:(h + 1) * DV, :]
                   * g_v[:, None].astype(np.float32)).astype(bf)  # [dv, dim]
        in_maps.append({
            "xT": xT,
            "w_all": w_all,
            "biasT": biasT,
            "w_out": w_out_h,
            "gq": (g_q * SCALE).astype(np.float32).reshape(DQK, 1),
            "gk": g_k.astype(np.float32).reshape(DQK, 1),
        })
    return in_maps


def kernel(x, attn_bias, w_qkv, w_out, g_q, g_k, g_v):
    x = np.asarray(x, dtype=np.float32)
    attn_bias = np.asarray(attn_bias, dtype=np.float32)
    w_qkv = np.asarray(w_qkv, dtype=np.float32)
    w_out = np.asarray(w_out, dtype=np.float32)
    g_q = np.asarray(g_q, dtype=np.float32)
    g_k = np.asarray(g_k, dtype=np.float32)
    g_v = np.asarray(g_v, dtype=np.float32)

    b, n, dim = x.shape
    nc = _get_nc(b, n, dim)
    in_maps = make_in_maps(x, attn_bias, w_qkv, w_out, g_q, g_k, g_v)
    res = run_bass_kernel_spmd(nc, in_maps, core_ids=list(range(N_CORES)),
                               trace=os.environ.get("KERNEL_TRACE", "") not in ("", "0"))
    _LAST_STATS["exec_time_ns"] = res.exec_time_ns
    _LAST_STATS["mean_exec_time_ns"] = res.mean_exec_time_ns
    _LAST_STATS["res"] = res
    out = np.zeros((b, n, dim), dtype=np.float32)
    for c in range(N_CORES):
        out += res.results[c]["out"].astype(np.float32)
    return out
